# revision 1
# baseline (speedup 1.0000x reference)
"""Trainium2 Bass kernel for nn_DiffConvAdaptive (B=32, N=1024, C=768, K=3).

Sharding: data-parallel over batch, 8 cores x 4 samples, no collectives.

Per-core pipeline (B_loc=4, N=1024=32x32, C=768):
  1. adaptive_avg_pool1d commutes with the p1 linear layer, so we pool raw
     x with a precomputed (1024, 9) segment matrix S: xp = S.T @ x (PE).
  2. t = silu(xp @ p1_w.T + p1_b)            (9 rows/sample - tiny)
  3. k = kg_w_eff @ t + kg_b_eff, where kg_w_eff folds the
     "kernels - sigmoid(beta)*mean(kernels)" correction on the host
     (sigmoid(beta) is uniform across channels; beta is zeros).
  4. x1 = x @ p2_w.T + p2_b produced channel-major [C, N] directly into a
     zero-padded 34x34 conv workspace (34-el row stride).
  5. depthwise 3x3 conv on the PE: per chunk, one DVE multiply builds all
     nine per-channel diagonal matrices (eye9x * broadcast(kT)); each tap
     is a diagonal-stationary matmul whose moving operand is a strided AP
     into the padded image, accumulating in fp32 PSUM. The program runs in
     two phases: all kernel-generation chains first, then the dense
     p2 -> conv -> proj PE stream (keeps the PE clock warm).
  6. out = conv @ proj_w.T + proj_b: channel-major conv tiles are the
     stationary matmul operand so output is token-major (contiguous DMA);
     biases enter PSUM via K=1 ones-row matmuls.

All matmuls bf16 (PE 1 cyc/row), fp32 PSUM accumulation, fp32 output.
"""

import sys

if "/opt/trn_rl_repo" not in sys.path:
    sys.path.insert(0, "/opt/trn_rl_repo")

import numpy as np
import ml_dtypes

import concourse.bass as bass
import concourse.bacc as bacc
import concourse.mybir as mybir
import concourse.tile as tile
from concourse.bass_utils import run_bass_kernel_spmd

N_CORES = 8
B, N, C = 32, 1024, 768
B_LOC = B // N_CORES
KK = 9
NCH = C // 128   # 6 channel chunks
NTC = N // 128   # 8 token chunks

# conv workspace: 34x34 padded image per partition row (stride 34), pixel
# (y, x) at offset 34*(y+1) + (x+1), zero pad ring. The depthwise conv runs
# on the PE as 9 diagonal-stationary matmuls accumulating in PSUM: tap
# (dy, dx) reads the moving operand with AP [[34,16],[1,32]] from offset
# 34*dy + dx (+ 544 for the second half) -- pad cells supply the zeros.
WS = 1160

BF = mybir.dt.bfloat16
F32 = mybir.dt.float32

_CACHE = {}
LAST_RESULTS = None


def _segment_matrix():
    S = np.zeros((N, KK), np.float32)
    for i in range(KK):
        s = (i * N) // KK
        e = -((-(i + 1) * N) // KK)
        S[s:e, i] = 1.0 / (e - s)
    return S


def _rows(ap_tile, base, nrows, ncols, c0):
    """[128, nrows, ncols] view of a workspace tile: rows at stride 36
    starting at offset base, columns c0..c0+ncols within each row."""
    v = ap_tile[:, base:base + 36 * nrows]
    v = v.rearrange("p (r e) -> p r e", e=36)
    return v[:, :, c0:c0 + ncols]


def build_program():
    # Bacc: its lowering legalizes multi-sem waits (splits drains etc.)
    # that walrus rejects when emitted raw from TileContext on bass.Bass
    nc = bacc.Bacc(None)

    x_d = nc.dram_tensor("xbf", [B_LOC, N, C], BF, kind="ExternalInput")
    wp2T_d = nc.dram_tensor("wp2T", [C, C], BF, kind="ExternalInput")
    wp1T_d = nc.dram_tensor("wp1T", [C, C], BF, kind="ExternalInput")
    wprojT_d = nc.dram_tensor("wprojT", [C, C], BF, kind="ExternalInput")
    S_d = nc.dram_tensor("S", [N, KK], BF, kind="ExternalInput")
    kgT_d = nc.dram_tensor("kgT", [KK, KK], BF, kind="ExternalInput")
    p1b_d = nc.dram_tensor("p1b", [1, C], BF, kind="ExternalInput")
    projb_d = nc.dram_tensor("projb", [1, C], BF, kind="ExternalInput")
    p2bT_d = nc.dram_tensor("p2bT", [C, 1], F32, kind="ExternalInput")
    kgb_d = nc.dram_tensor("kgb", [KK, 1], F32, kind="ExternalInput")
    ones_d = nc.dram_tensor("ones", [1, 128], BF, kind="ExternalInput")
    eye9f_d = nc.dram_tensor("eye9f", [KK, KK], F32, kind="ExternalInput")
    eye128b_d = nc.dram_tensor("eye128b", [128, 128], BF, kind="ExternalInput")
    eye9x_d = nc.dram_tensor("eye9x", [128, KK * 128], BF, kind="ExternalInput")
    out_d = nc.dram_tensor("out", [B_LOC, N, C], F32, kind="ExternalOutput")

    with tile.TileContext(nc) as tc:
        with (
            tc.tile_pool(name="const", bufs=1) as cpool,
            tc.tile_pool(name="ws", bufs=1) as wspool,
            tc.tile_pool(name="io", bufs=3) as iopool,
            tc.tile_pool(name="xt", bufs=12) as xtpool,
            tc.tile_pool(name="cv", bufs=26) as cvpool,
            tc.tile_pool(name="co", bufs=12) as copool,
            tc.tile_pool(name="kgen", bufs=2) as kgpool,
            tc.tile_pool(name="ktp", bufs=12) as ktpool,
            tc.tile_pool(name="psA", bufs=3, space="PSUM") as psA,
            tc.tile_pool(name="psB", bufs=3, space="PSUM") as psB,
            tc.tile_pool(name="psS", bufs=2, space="PSUM") as psS,
        ):
            # ---------------- constants ----------------
            wp2T, wp1T, wprojT = [], [], []
            for i in range(NCH):
                t2 = cpool.tile([128, C], BF, tag=f"wp2T{i}")
                nc.sync.dma_start(t2[:], wp2T_d[128 * i:128 * (i + 1), :])
                wp2T.append(t2)
                t1 = cpool.tile([128, C], BF, tag=f"wp1T{i}")
                nc.sync.dma_start(t1[:], wp1T_d[128 * i:128 * (i + 1), :])
                wp1T.append(t1)
                tp = cpool.tile([128, C], BF, tag=f"wprojT{i}")
                nc.sync.dma_start(tp[:], wprojT_d[128 * i:128 * (i + 1), :])
                wprojT.append(tp)

            S_sb = cpool.tile([128, NTC * KK], BF, tag="S")
            for t in range(NTC):
                nc.sync.dma_start(
                    S_sb[:, KK * t:KK * (t + 1)], S_d[128 * t:128 * (t + 1), :]
                )
            kgT_sb = cpool.tile([KK, KK], BF, tag="kgT")
            nc.sync.dma_start(kgT_sb[:], kgT_d[:])
            p1b_sb = cpool.tile([1, C], BF, tag="p1b")
            nc.sync.dma_start(p1b_sb[:], p1b_d[:])
            projb_sb = cpool.tile([1, C], BF, tag="projb")
            nc.sync.dma_start(projb_sb[:], projb_d[:])
            p2bT_sb = cpool.tile([128, NCH], F32, tag="p2bT")
            for i in range(NCH):
                nc.sync.dma_start(p2bT_sb[:, i:i + 1], p2bT_d[128 * i:128 * (i + 1), :])
            kgb_sb = cpool.tile([KK, 1], F32, tag="kgb")
            nc.sync.dma_start(kgb_sb[:], kgb_d[:])
            ones_sb = cpool.tile([1, 128], BF, tag="ones")
            nc.sync.dma_start(ones_sb[:], ones_d[:])
            eye9f = cpool.tile([KK, KK], F32, tag="eye9f")
            nc.sync.dma_start(eye9f[:], eye9f_d[:])
            eye128b = cpool.tile([128, 128], BF, tag="eye128b")
            nc.sync.dma_start(eye128b[:], eye128b_d[:])
            eye9x = cpool.tile([128, KK * 128], BF, tag="eye9x")
            nc.sync.dma_start(eye9x[:], eye9x_d[:])

            # conv workspaces: pad ring zeroed once, interior overwritten
            x1t = []
            for i in range(NCH):
                a = wspool.tile([128, WS], BF, tag=f"x1t{i}")
                nc.gpsimd.memset(a[:], 0.0)
                x1t.append(a)

            add = mybir.AluOpType.add

            dg_all = {}
            # ---- phase 1: per-sample kernel-generation chains ----
            # (latency-bound ACT/DVE work; keeps PE-light so phase 2 can
            # run the dense p2+conv+proj stream without cold-clock gaps)
            for b in range(B_LOC):
                # ---- pooling (token-major loads) ----
                pp = [psS.tile([KK, 384], F32, tag="pss", name=f"pp{b}_{_h}") for _h in range(2)]
                for t in range(NTC):
                    xn = iopool.tile([128, C], BF, tag="xn")
                    nc.sync.dma_start(xn[:], x_d[b, 128 * t:128 * (t + 1), :])
                    for h in range(2):
                        nc.tensor.matmul(
                            pp[h][:],
                            S_sb[:, KK * t:KK * (t + 1)],
                            xn[:, 384 * h:384 * (h + 1)],
                            start=(t == 0),
                            stop=(t == NTC - 1),
                        )
                xp = kgpool.tile([KK, C], BF, tag="xp")
                for h in range(2):
                    nc.vector.tensor_copy(xp[:, 384 * h:384 * (h + 1)], pp[h][:])

                # xp -> xpT chunks [128, 9]
                xpT = []
                for i in range(NCH):
                    tp = psS.tile([128, KK], BF, tag="pss")
                    nc.tensor.transpose(tp[:], xp[:, 128 * i:128 * (i + 1)], eye128b[:KK, :KK])
                    sb = ktpool.tile([128, KK], BF, tag="xpT")
                    nc.vector.tensor_copy(sb[:], tp[:])
                    xpT.append(sb)

                # p1 + silu
                tsil = kgpool.tile([KK, C], BF, tag="tsil")
                for h in range(2):
                    tp1 = psS.tile([KK, 384], F32, tag="pss")
                    nc.tensor.matmul(
                        tp1[:], ones_sb[:1, :KK],
                        p1b_sb[:1, 384 * h:384 * (h + 1)],
                        start=True, stop=False,
                    )
                    for i in range(NCH):
                        nc.tensor.matmul(
                            tp1[:], xpT[i][:],
                            wp1T[i][:, 384 * h:384 * (h + 1)],
                            start=False, stop=(i == NCH - 1),
                        )
                    # silu(v) = v * sigmoid(v); Silu LUT is absent in CoreSim
                    sg = kgpool.tile([KK, 384], BF, tag="sg")
                    nc.scalar.activation(
                        sg[:], tp1[:], mybir.ActivationFunctionType.Sigmoid,
                    )
                    nc.vector.tensor_tensor(
                        tsil[:, 384 * h:384 * (h + 1)], tp1[:], sg[:],
                        mybir.AluOpType.mult,
                    )

                # kernel gen
                ksb = kgpool.tile([KK, C], F32, tag="ksb")
                for h in range(2):
                    kp = psS.tile([KK, 384], F32, tag="pss")
                    nc.tensor.matmul(
                        kp[:], kgT_sb[:], tsil[:, 384 * h:384 * (h + 1)],
                        start=True, stop=True,
                    )
                    nc.scalar.activation(
                        ksb[:, 384 * h:384 * (h + 1)], kp[:],
                        mybir.ActivationFunctionType.Identity,
                        bias=kgb_sb[:],
                    )

                # k -> kT chunks [128, 9] f32 (conv scalars)
                kT = []
                for i in range(NCH):
                    tp = psS.tile([128, KK], F32, tag="pss")
                    nc.tensor.transpose(tp[:], ksb[:, 128 * i:128 * (i + 1)], eye9f[:])
                    sb = ktpool.tile([128, KK], BF, tag="kT")
                    nc.vector.tensor_copy(sb[:], tp[:])
                    kT.append(sb)

                # all 9 diag matrices per chunk: eye9x * broadcast(kT)
                for i in range(NCH):
                    dg = cvpool.tile([128, KK * 128], BF, tag="dg",
                                     name=f"dg{b}_{i}")
                    kbc = kT[i][:].broadcast_to((128, KK, 128))
                    nc.vector.tensor_tensor(
                        dg[:].rearrange("p (j f) -> p j f", f=128),
                        eye9x[:].rearrange("p (j f) -> p j f", f=128),
                        kbc,
                        mybir.AluOpType.mult,
                    )
                    dg_all[(b, i)] = dg

            # ---- phase 2: dense PE stream (p2 -> conv -> proj) ----
            for b in range(B_LOC):
                xT = []
                for i in range(NCH):
                    tt = xtpool.tile([128, N], BF, tag="xT", name=f"xT{b}_{i}")
                    nc.sync.dma_start(
                        tt[:], x_d[b, :, 128 * i:128 * (i + 1)], transpose=True
                    )
                    xT.append(tt)
                conv = []
                for i in range(NCH):
                    xps = [psA.tile([128, 512], F32, tag="psa",
                                    name=f"xps{b}_{i}_{_h}") for _h in range(2)]
                    for kc in range(NCH):
                        for h in range(2):
                            nc.tensor.matmul(
                                xps[h][:],
                                wp2T[kc][:, 128 * i:128 * (i + 1)],
                                xT[kc][:, 512 * h:512 * (h + 1)],
                                start=(kc == 0),
                                stop=(kc == NCH - 1),
                            )
                    for h in range(2):
                        # evacuate into padded rows (+bias, ->bf16)
                        rb = 34 * (1 + 16 * h)
                        dst = x1t[i][:, rb:rb + 544]
                        dst = dst.rearrange("p (r e) -> p r e", e=34)[:, :, 1:33]
                        nc.scalar.activation(
                            dst,
                            xps[h][:].rearrange("p (r e) -> p r e", e=32),
                            mybir.ActivationFunctionType.Identity,
                            bias=p2bT_sb[:, i:i + 1],
                        )
                    dg = dg_all[(b, i)]
                    # conv: 9 diag-stationary matmuls per half, PSUM accum
                    cv = copool.tile([128, N], BF, tag="conv")
                    pc = [psA.tile([128, 512], F32, tag="psa",
                                   name=f"pc{b}_{i}_{_h}") for _h in range(2)]
                    for dy in range(3):
                        for dx in range(3):
                            j = 3 * dy + dx
                            for h in range(2):
                                base = 34 * dy + dx + 544 * h
                                rhs = x1t[i][:, base:base + 544]
                                rhs = rhs.rearrange("p (r e) -> p r e", e=34)[:, :, :32]
                                nc.tensor.matmul(
                                    pc[h][:],
                                    dg[:, 128 * j:128 * (j + 1)],
                                    rhs,
                                    start=(j == 0),
                                    stop=(j == 8),
                                )
                    for h in range(2):
                        nc.vector.tensor_copy(cv[:, 512 * h:512 * (h + 1)], pc[h][:])
                    conv.append(cv)

                # ---------------- proj ----------------
                for t in range(NTC):
                    po = [psB.tile([128, 384], F32, tag="psb",
                                   name=f"po{b}_{t}_{_h}") for _h in range(2)]
                    for h in range(2):
                        nc.tensor.matmul(
                            po[h][:], ones_sb[:1, :],
                            projb_sb[:1, 384 * h:384 * (h + 1)],
                            start=True, stop=False,
                        )
                    for kc in range(NCH):
                        for h in range(2):
                            nc.tensor.matmul(
                                po[h][:],
                                conv[kc][:, 128 * t:128 * (t + 1)],
                                wprojT[kc][:, 384 * h:384 * (h + 1)],
                                start=False, stop=(kc == NCH - 1),
                            )
                    for h in range(2):
                        osb = iopool.tile([128, 384], F32, tag="osb")
                        nc.vector.tensor_copy(osb[:], po[h][:])
                        nc.sync.dma_start(
                            out_d[b, 128 * t:128 * (t + 1), 384 * h:384 * (h + 1)],
                            osb[:],
                        )
    nc.finalize()
    return nc


def _prepare_weights(inputs):
    bf = ml_dtypes.bfloat16
    p1_w = np.asarray(inputs["p1_w"], np.float32)
    p1_b = np.asarray(inputs["p1_b"], np.float32)
    kg_w = np.asarray(inputs["kg_w"], np.float32)
    kg_b = np.asarray(inputs["kg_b"], np.float32)
    p2_w = np.asarray(inputs["p2_w"], np.float32)
    p2_b = np.asarray(inputs["p2_b"], np.float32)
    proj_w = np.asarray(inputs["proj_w"], np.float32)
    proj_b = np.asarray(inputs["proj_b"], np.float32)
    beta = np.asarray(inputs["beta"], np.float32)

    factor = 1.0 / (1.0 + np.exp(-beta))
    assert np.allclose(factor, factor[0], atol=1e-6), (
        "non-uniform sigmoid(beta) not supported by the host fold"
    )
    A = np.eye(KK, dtype=np.float32) - float(factor[0]) / KK
    kg_w_eff = (A @ kg_w).astype(np.float32)
    kg_b_eff = (A @ kg_b).astype(np.float32)

    return {
        "wp2T": np.ascontiguousarray(p2_w.T).astype(bf),
        "wp1T": np.ascontiguousarray(p1_w.T).astype(bf),
        "wprojT": np.ascontiguousarray(proj_w.T).astype(bf),
        "S": _segment_matrix().astype(bf),
        "kgT": np.ascontiguousarray(kg_w_eff.T).astype(bf),
        "p1b": p1_b.reshape(1, C).astype(bf),
        "projb": proj_b.reshape(1, C).astype(bf),
        "p2bT": np.ascontiguousarray(p2_b.reshape(C, 1)),
        "kgb": np.ascontiguousarray(kg_b_eff.reshape(KK, 1)),
        "ones": np.ones((1, 128), bf),
        "eye9f": np.eye(KK, dtype=np.float32),
        "eye128b": np.eye(128, dtype=np.float32).astype(bf),
        "eye9x": np.ascontiguousarray(
            np.tile(np.eye(128, dtype=np.float32), (1, KK))
        ).astype(bf),
    }


def kernel(**inputs):
    global LAST_RESULTS
    if "nc" not in _CACHE:
        _CACHE["nc"] = build_program()
    nc = _CACHE["nc"]

    x = np.asarray(inputs["x"], np.float32)
    weights = _prepare_weights(inputs)
    xbf = x.astype(ml_dtypes.bfloat16)

    in_maps = []
    for c in range(N_CORES):
        m = dict(weights)
        m["xbf"] = np.ascontiguousarray(xbf[B_LOC * c:B_LOC * (c + 1)])
        in_maps.append(m)

    res = run_bass_kernel_spmd(nc, in_maps, list(range(N_CORES)))
    LAST_RESULTS = res
    out = np.concatenate([res.results[c]["out"] for c in range(N_CORES)], axis=0)
    return np.ascontiguousarray(out.astype(np.float32))

